# revision 23
# baseline (speedup 1.0000x reference)
"""AdvancedFeatureTransformer Trainium2 kernel (v2).

Data-parallel over batch (8 cores x 512 rows), no collectives. Activations
feature-major (h^T: [feat_part, batch_free]) so every matmul streams the
batch as the N=512 moving operand.

v2 vs v1 (baseline was 6.28 ms, GPSIMD-bound at 86% with fp32 matmuls):
- all matmuls run as float32r (bitcast views): 1 cycle/row at N>=256 vs 4
  cycles/row for plain fp32 on the PE.
- attention v/out projections fold on host into one (Wo@Wv + I) matmul with
  the residual identity included; LN centering applied to the folded matrix,
  which also removes the first-layer mean-correction machinery. Same fold
  for cross-attention.
- trunk LN-input biases enter via K=1 ones-row matmul accumulation into
  PSUM, so LN consumers read PSUM directly (no bias op, no y materialize).
- zero GPSIMD elementwise (each op measured 7.5us there): relu/square/
  scale split across ACT (psum reads, bias fused) and DVE.
- head weight DMAs batched: W1 per 2 pairs, W2 per 4 pairs, W3 per group.
- ACT functions chosen from one table set (sqrt_and_others) in the heads:
  Square/Sqrt/Relu/Identity -> no ACT table reloads in the hot loop.

Per-pair head pipeline (targets a,b; feature-major [128f, 512b]):
  PE:  ps_a, ps_b = W1c^T @ hc (2 matmuls each)
  ACT: sq_e = Square(ps_e + b1c_e)          DVE: R_e = max(ps_e + b1c_e, 0)
  PE:  stp[0:64] = ones64^T sq_a ; stp[64:128] = ones64^T sq_b  (replicated)
  ACT: sd = Sqrt(stp/128 + eps)             DVE: rb = 1/sd (approx)
  PE:  zps = W2pair^T @ [R_a|R_b]
  DVE: U = zps * rb                         ACT: R2 = Relu(U + b2)
  PE:  o3g += blockdiag(W3)^T @ R2  (accumulate over 16 pairs)
"""

import sys

if "/opt/trn_rl_repo" not in sys.path:
    sys.path.insert(0, "/opt/trn_rl_repo")

import numpy as np

B = 4096
NCORES = 8
BL = B // NCORES        # 512 rows per core
DIN = 512
D = 256
T = 424
L = 6
EPS = 1e-5
PAIRS = T // 2          # 212

_cache = {}


def _prep(inputs):
    """Host-side weight preprocessing -> per-core input map (shared arrays)."""
    f32 = lambda a: np.ascontiguousarray(np.asarray(a, dtype=np.float32))
    f64 = lambda a: np.asarray(a, dtype=np.float64)

    x = f32(inputs["x"])
    # trunk LN scale/bias must be identity for the zero-mean centering trick
    assert np.all(np.asarray(inputs["ln_g"]) == 1.0), "ln_g != 1 unsupported"
    assert np.all(np.asarray(inputs["ln_b"]) == 0.0), "ln_b != 0 unsupported"
    assert np.all(np.asarray(inputs["tp_ln_g"]) == 1.0), "tp_ln_g != 1 unsupported"
    assert np.all(np.asarray(inputs["tp_ln_b"]) == 0.0), "tp_ln_b != 0 unsupported"

    # ---- projection ----
    Wp = f32(inputs["proj_W"]).reshape(D, DIN)        # [256, 512]
    bp = f32(inputs["proj_b"]).reshape(D)
    WpT = f32(Wp.T)                                   # [512, 256]

    # ---- trunk layers: fold attn (= out(v(h)) since seq_len==1) ----
    aiW = f64(inputs["attn_in_W"])                    # [6, 768, 256]
    aib = f64(inputs["attn_in_b"])                    # [6, 768]
    aoW = f64(inputs["attn_out_W"])                   # [6, 256, 256]
    aob = f64(inputs["attn_out_b"])
    f1W = f32(inputs["ff_W1"])                        # [6, 1024, 256]
    f1b = f32(inputs["ff_b1"])
    f2W = f32(inputs["ff_W2"])                        # [6, 256, 1024]
    f2b = f32(inputs["ff_b2"])

    AT = np.empty((L, D, D), np.float32)              # lhsT of (Wo@Wv + I)_c
    baC = np.empty((L, D), np.float32)
    Wf1T = np.empty((L, D, 4 * D), np.float32)
    Wf2T = np.empty((L, 4 * D, D), np.float32)
    bf2C = np.empty((L, D), np.float32)
    for i in range(L):
        Wv, bv = aiW[i, 2 * D:], aib[i, 2 * D:]
        A = aoW[i] @ Wv + np.eye(D)                   # residual folded in
        ba = aoW[i] @ bv + aob[i]
        A = A - A.mean(0, keepdims=True)              # exact zero feature-mean
        ba = ba - ba.mean()
        AT[i] = f32(A.T)
        baC[i] = f32(ba)
        Wf1T[i] = f1W[i].T
        Wf2 = f2W[i] - f2W[i].mean(0, keepdims=True)
        Wf2T[i] = Wf2.T
        bf2C[i] = f32(f2b[i] - f2b[i].mean())
    bf1 = f32(f1b)                                    # [6, 1024]

    # ---- cross attention fold (no LN after -> no centering) ----
    cW = f64(inputs["cross_in_W"])
    coW = f64(inputs["cross_out_W"])
    AC = coW @ cW[2 * D:] + np.eye(D)
    bac = coW @ f64(inputs["cross_in_b"])[2 * D:] + f64(inputs["cross_out_b"])
    ACT_ = f32(AC.T)                                  # [256, 256]
    bacC = f32(bac)

    # ---- column-bias pack (ACT bias columns): one [nb,128] -> sbuf [128,nb]
    cols = []

    def pack(vec):
        v = f32(vec).reshape(-1, 128)
        s = len(cols)
        cols.extend(v)
        return s

    bias_idx = {
        "bp": pack(bp),
        "bf1": [pack(bf1[i]) for i in range(L)],
        "bac": pack(bacC),
    }
    TB = f32(np.stack(cols))                           # [nb, 128]

    # ---- row-bias pack (K=1 matmul lhsT rows): [1, nr*128] ----
    rows = []

    def rpack(vec):
        v = f32(vec).reshape(-1, 128)
        s = len(rows)
        rows.extend(v)
        return s

    row_idx = {
        "ba": [rpack(baC[i]) for i in range(L)],
        "bf2": [rpack(bf2C[i]) for i in range(L)],
    }
    import ml_dtypes
    bf16 = lambda a: np.ascontiguousarray(np.asarray(a, dtype=ml_dtypes.bfloat16))
    RB = bf16(np.concatenate(rows).reshape(1, -1))     # [1, nr*128]

    # ---- heads ----
    W1 = f32(inputs["tp_W1"])                          # [424, 128, 256]
    b1 = f32(inputs["tp_b1"])                          # [424, 128]
    W1c = W1 - W1.mean(1, keepdims=True)
    b1c = b1 - b1.mean(1, keepdims=True)
    W1T = W1c.transpose(0, 2, 1)                       # [424, 256, 128]

    W2 = f32(inputs["tp_W2"])                          # [424, 64, 128]
    b2 = f32(inputs["tp_b2"])                          # [424, 64]
    W2P = W2.transpose(0, 2, 1).reshape(PAIRS, 2, 128, 64)
    b2P = f32(b2.reshape(PAIRS, 128).T)                # [128, 212]

    W3 = f32(inputs["tp_W3"])                          # [424, 64]
    b3 = f32(inputs["tp_b3"])                          # [424]
    NG = (T + 31) // 32                                # 14 groups of <=32
    W3BD = np.zeros((PAIRS, 128, 32), np.float32)
    for p in range(PAIRS):
        q = p % 16                                     # pair index in group
        W3BD[p, 0:64, 2 * q] = W3[2 * p]
        W3BD[p, 64:128, 2 * q + 1] = W3[2 * p + 1]
    b3B = np.zeros((32, NG), np.float32)
    for t in range(T):
        b3B[t % 32, t // 32] = b3[t]

    shared = {
        "WpT": WpT, "AT": AT, "Wf1T": Wf1T, "Wf2T": Wf2T, "ACT_": ACT_,
        "TB": TB, "RB": RB,
        "W1T": bf16(W1T), "b1T": f32(b1c.T), "W2P": bf16(W2P),
        "b2P": b2P,
        "W3BD": bf16(W3BD), "b3B": b3B,
    }
    in_maps = []
    for c in range(NCORES):
        m = dict(shared)
        m["xT"] = f32(x[c * BL:(c + 1) * BL].T)        # [512 din, 512 b]
        in_maps.append(m)
    return in_maps, TB.shape[0], RB.shape[1], bias_idx, row_idx


def _build(nb, nrb, bias_idx, row_idx):
    import concourse.bass as bass
    import concourse.mybir as mybir
    import concourse.tile as tile
    from concourse import bacc
    from concourse.masks import make_identity

    dt = mybir.dt.float32
    f32r = mybir.dt.float32r
    bf = mybir.dt.bfloat16
    Alu = mybir.AluOpType
    Act = mybir.ActivationFunctionType
    ts = bass.ts

    nc = bacc.Bacc(None, target_bir_lowering=False)

    mmr = nc.tensor.matmul   # operands declared float32r (1 cyc/row, N>=256)

    dr = lambda name, shape, d=dt: nc.dram_tensor(name, shape, d,
                                                  kind="ExternalInput")
    xT = dr("xT", [DIN, BL], f32r)
    WpT = dr("WpT", [DIN, D], f32r)
    AT = dr("AT", [L, D, D], f32r)
    Wf1T = dr("Wf1T", [L, D, 4 * D], f32r)
    Wf2T = dr("Wf2T", [L, 4 * D, D], f32r)
    ACT_ = dr("ACT_", [D, D], f32r)
    TB = dr("TB", [nb, 128])
    RB = dr("RB", [1, nrb], bf)
    W1T = dr("W1T", [T, D, 128], bf)
    b1T = dr("b1T", [128, T])
    W2P = dr("W2P", [PAIRS, 2, 128, 64], bf)
    b2P = dr("b2P", [128, PAIRS])
    W3BD = dr("W3BD", [PAIRS, 128, 32], bf)
    b3B = dr("b3B", [32, 14])
    out = nc.dram_tensor("out", [BL, T], dt, kind="ExternalOutput")

    from contextlib import ExitStack

    with tile.TileContext(nc) as tc, ExitStack() as stack:
        consts = stack.enter_context(tc.tile_pool(name="consts", bufs=1))
        hpool = stack.enter_context(tc.tile_pool(name="hpool", bufs=4))

        # constants (f32r tiles are round-copied from fp32 scratch: memset /
        # make_identity cannot target float32r directly)
        onesf = consts.tile([128, 128], dt, tag="onesf")
        nc.vector.memset(onesf, 1.0)
        onesrf = consts.tile([1, BL], dt, tag="onesrf")
        nc.vector.memset(onesrf, 1.0)
        idnf = consts.tile([128, 128], dt, tag="idnf")
        make_identity(nc, idnf)
        ones_col = consts.tile([128, 1], bf, tag="ones")
        nc.vector.tensor_copy(out=ones_col, in_=onesf[:, 0:1])
        ones_row = consts.tile([1, BL], bf, tag="onesr")
        nc.vector.tensor_copy(out=ones_row, in_=onesrf)
        eps_col = consts.tile([128, 1], dt, tag="eps")
        nc.vector.memset(eps_col, EPS)
        ones64 = consts.tile([128, 64], bf, tag="ones64")
        nc.vector.tensor_copy(out=ones64, in_=onesf[:, 0:64])
        ones128 = consts.tile([128, 128], bf, tag="ones128")
        nc.vector.tensor_copy(out=ones128, in_=onesf)
        idn = consts.tile([128, 128], f32r, tag="idn")
        nc.vector.tensor_copy(out=idn, in_=idnf)
        tb_sb = consts.tile([128, nb], dt, tag="tb")
        nc.gpsimd.dma_start(out=tb_sb, in_=TB.rearrange("n p -> p n"))
        rb_sb = consts.tile([1, nrb], bf, tag="rb")
        b1_sb = consts.tile([128, T], dt, tag="b1c")
        nc.gpsimd.dma_start(out=b1_sb, in_=b1T[:, :])
        nc.gpsimd.dma_start(out=rb_sb, in_=RB[:, :])

        b2_sb = consts.tile([128, PAIRS], dt, tag="b2")
        nc.gpsimd.dma_start(out=b2_sb, in_=b2P[:, :])
        b3_sb = consts.tile([32, 14], dt, tag="b3")
        nc.gpsimd.dma_start(out=b3_sb, in_=b3B[:, :])
        out_sb = [consts.tile([128, T], dt, tag=f"ob{i}", name=f"ob{i}")
                  for i in range(4)]

        def bias_col(idx, m=0):
            return tb_sb[:, idx + m:idx + m + 1]

        def brow(idx, m=0):
            j = (idx + m) * 128
            return rb_sb[0:1, j:j + 128]

        # ---------------- trunk ----------------
        with tc.tile_pool(name="twt", bufs=2) as twt, \
             tc.tile_pool(name="tact", bufs=4) as tact, \
             tc.tile_pool(name="tps", bufs=4, space="PSUM") as tps, \
             tc.tile_pool(name="tpss", bufs=2, space="PSUM") as tpss:

            xs = twt.tile([128, 4, BL], f32r, tag="x")
            nc.scalar.dma_start(out=xs, in_=xT.rearrange("(c k) b -> k c b", c=4))
            wp = twt.tile([128, 4, D], f32r, tag="wp")
            nc.scalar.dma_start(out=wp, in_=WpT.rearrange("(c k) m -> k c m", c=4))

            h = [hpool.tile([128, BL], f32r, tag=f"h{m}", name=f"h{m}")
                 for m in range(2)]
            for m in range(2):
                ps = tps.tile([128, BL], dt, tag="mm")
                for k in range(4):
                    mmr(ps, wp[:, k, ts(m, 128)], xs[:, k],
                        start=(k == 0), stop=(k == 3))
                nc.scalar.activation(h[m], ps, Act.Identity,
                                     bias=bias_col(bias_idx["bp"], m), scale=1.0)

            def layer_norm(pss):
                """pss: 2 feature-major [128,BL] PSUM tiles holding the
                (exactly zero-mean) LN input incl. bias. Returns hn tiles.
                Stats matmul uses an all-ones [128,128] lhsT so the sum of
                squares lands replicated on every partition -> rstd comes
                from one ACT op, no cross-partition broadcast needed."""
                stp = tpss.tile([128, BL], dt, tag="ssq")
                for k in range(2):
                    sq = tact.tile([128, BL], bf, tag="sq")
                    nc.scalar.activation(sq, pss[k], Act.Square, bias=0.0,
                                         scale=1.0)
                    mmr(stp, ones128, sq, start=(k == 0), stop=(k == 1))
                rbb = tact.tile([128, BL], dt, tag="rbb")
                nc.scalar.activation(rbb, stp, Act.Abs_reciprocal_sqrt,
                                     bias=eps_col, scale=1.0 / D)
                hn = [hpool.tile([128, BL], f32r, tag=f"h{m}", name=f"hn{m}")
                      for m in range(2)]
                for m in range(2):
                    nc.vector.tensor_tensor(out=hn[m], in0=pss[m], in1=rbb,
                                            op=Alu.mult)
                return hn

            for i in range(L):
                # attention + residual: y = (Wo Wv + I)_c h + ba_c
                wa = twt.tile([128, 2, D], f32r, tag="wa")
                nc.gpsimd.dma_start(out=wa, in_=AT[i].rearrange(
                    "(c k) m -> k c m", c=2))
                pss = []
                for m in range(2):
                    ps = tps.tile([128, BL], dt, tag="mm", name=f"y{m}")
                    for k in range(2):
                        mmr(ps, wa[:, k, ts(m, 128)], h[k],
                            start=(k == 0), stop=False)
                    mmr(ps, brow(row_idx["ba"][i], m), ones_row,
                        start=False, stop=True)
                    pss.append(ps)
                h = layer_norm(pss)

                # feed-forward
                w1 = twt.tile([128, 2, 4 * D], f32r, tag="wf1")
                nc.gpsimd.dma_start(out=w1, in_=Wf1T[i].rearrange(
                    "(c k) m -> k c m", c=2))
                g = []
                for m in range(8):
                    ps = tps.tile([128, BL], dt, tag="mm")
                    for k in range(2):
                        mmr(ps, w1[:, k, ts(m, 128)], h[k],
                            start=(k == 0), stop=(k == 1))
                    gt = tact.tile([128, BL], f32r, tag=f"g{m}", name=f"g{m}")
                    nc.scalar.activation(gt, ps, Act.Gelu,
                                         bias=bias_col(bias_idx["bf1"][i], m),
                                         scale=1.0)
                    g.append(gt)
                w2 = twt.tile([128, 8, D], f32r, tag="wf2")
                nc.gpsimd.dma_start(out=w2, in_=Wf2T[i].rearrange(
                    "(c k) m -> k c m", c=8))
                pss = []
                for m in range(2):
                    ps = tps.tile([128, BL], dt, tag="mm", name=f"y2{m}")
                    for k in range(8):
                        mmr(ps, w2[:, k, ts(m, 128)], g[k],
                            start=(k == 0), stop=False)
                    mmr(ps, idn, h[m], start=False, stop=False)  # + residual
                    mmr(ps, brow(row_idx["bf2"][i], m), ones_row,
                        start=False, stop=True)
                    pss.append(ps)
                h = layer_norm(pss)

            # cross attention fold: hc = (Wco Wcv + I) h + bac
            wc = twt.tile([128, 2, D], f32r, tag="wa")
            nc.gpsimd.dma_start(out=wc, in_=ACT_.rearrange(
                "(c k) m -> k c m", c=2))
            hc = [consts.tile([128, BL], bf, tag=f"hc{m}", name=f"hc{m}")
                  for m in range(2)]
            for m in range(2):
                ps = tps.tile([128, BL], dt, tag="mm")
                for k in range(2):
                    mmr(ps, wc[:, k, ts(m, 128)], h[k],
                        start=(k == 0), stop=(k == 1))
                nc.scalar.activation(hc[m], ps, Act.Identity,
                                     bias=bias_col(bias_idx["bac"], m),
                                     scale=1.0)

        # ---------------- heads (software-pipelined) ----------------
        # Stages per pair i: P = weight DMAs (lead 3), A = W1+bias matmuls ->
        # wide psum + Square/Relu issue, B = stats matmuls + arsqrt,
        # C = W2 matmuls + U + R2, D = W3 matmul (lag 2) + group/block flush.
        # The lags keep the PE stream free of dependency stalls.
        with tc.tile_pool(name="hw1", bufs=4) as hw1, \
             tc.tile_pool(name="hw2", bufs=3) as hw2, \
             tc.tile_pool(name="hw3", bufs=3) as hw3, \
             tc.tile_pool(name="hT", bufs=6) as hT, \
             tc.tile_pool(name="hS", bufs=3) as hS, \
             tc.tile_pool(name="hR", bufs=3) as hR, \
             tc.tile_pool(name="hrb", bufs=3) as hrb, \
             tc.tile_pool(name="hps", bufs=2, space="PSUM") as hps, \
             tc.tile_pool(name="hpw", bufs=1, space="PSUM") as hpw, \
             tc.tile_pool(name="hpst", bufs=1, space="PSUM") as hpst, \
             tc.tile_pool(name="hpo", bufs=1, space="PSUM") as hpo:

            sched = []
            for blk in range(4):
                bs = min(128, T - blk * 128)
                ngrp = (bs + 31) // 32
                for g in range(ngrp):
                    gs = min(32, bs - g * 32)
                    gg = blk * 4 + g
                    npair = gs // 2
                    for q in range(npair):
                        sched.append(dict(
                            blk=blk, bs=bs, gg=gg, g=g, q=q, npair=npair,
                            gs=gs, t0g=gg * 32, pg=gg * 16 + q,
                            fg=(q == 0), lg=(q == npair - 1),
                            fb=(g == 0 and q == 0),
                            lb=(g == ngrp - 1 and q == npair - 1)))
            NP = len(sched)
            st = [dict() for _ in range(NP)]
            grp = {}            # gg -> dict(b1g, w3g, o3g)
            blkst = {}          # blk -> o3 tile
            wcache = {}         # 'w1'//2 , 'w2'//4 -> tiles

            def stageP(i):
                e = sched[i]
                pg = e["pg"]
                if e["fg"]:
                    gd = grp.setdefault(e["gg"], {})
                    w3g = hw3.tile([128, 16, 32], bf, tag="w3", name="w3")
                    p0 = e["gg"] * 16
                    nc.gpsimd.dma_start(
                        out=w3g[:, 0:e["npair"]],
                        in_=W3BD[p0:p0 + e["npair"]].rearrange("p k m -> k p m"))
                    gd["w3g"] = w3g
                if pg % 2 == 0:
                    w1x = hw1.tile([128, 4, 2, 128], bf, tag="w1",
                                   name="w1")
                    tA = 2 * pg
                    nc.gpsimd.dma_start(
                        out=w1x, in_=W1T[tA:tA + 4].rearrange(
                            "t (c k) m -> k t c m", c=2))
                    wcache[("w1", pg // 2)] = w1x
                if pg % 4 == 0:
                    w2x = hw2.tile([128, 4, 2, 64], bf, tag="w2", name="w2")
                    np_ = min(4, PAIRS - pg)
                    nc.gpsimd.dma_start(
                        out=w2x[:, 0:np_], in_=W2P[pg:pg + np_]
                        .rearrange("p e k m -> k p e m"))
                    wcache[("w2", pg // 4)] = w2x

            def stageA(i):
                e = sched[i]
                pg, tA = e["pg"], 2 * e["pg"]
                w1x = wcache[("w1", pg // 2)]
                wide = hps.tile([128, 2 * BL], dt, tag="s", name="wide")
                sqw = hS.tile([128, 2 * BL], bf, tag="sq", name="sq")
                Rw = hR.tile([128, 2 * BL], bf, tag="R", name="R")
                for ei in range(2):
                    t = tA + ei
                    half = wide[:, ei * BL:(ei + 1) * BL]
                    for k in range(2):
                        mmr(half, w1x[:, 2 * (pg % 2) + ei, k], hc[k],
                            start=(k == 0), stop=(k == 1))
                    b1c = b1_sb[:, t:t + 1]
                    nc.scalar.activation(sqw[:, ei * BL:(ei + 1) * BL], half,
                                         Act.Square, bias=b1c, scale=1.0)
                    nc.vector.tensor_scalar(
                        out=Rw[:, ei * BL:(ei + 1) * BL], in0=half,
                        scalar1=b1c, scalar2=0.0, op0=Alu.add, op1=Alu.max)
                st[i] = dict(sqw=sqw, Rw=Rw)

            def stageB(i):
                s = st[i]
                stp = hpst.tile([128, BL], dt, tag="st", name="stp")
                mmr(stp[0:64], ones64, s["sqw"][:, 0:BL],
                    start=True, stop=True)
                mmr(stp[64:128], ones64, s["sqw"][:, BL:2 * BL],
                    start=True, stop=True)
                rb = hrb.tile([128, BL], dt, tag="rb", name="rb")
                nc.scalar.activation(rb, stp, Act.Abs_reciprocal_sqrt,
                                     bias=eps_col, scale=1.0 / 128)
                s["rb"] = rb

            def stageC(i):
                e = sched[i]
                s = st[i]
                pg = e["pg"]
                w2x = wcache[("w2", pg // 4)]
                zps = hpw.tile([128, BL], dt, tag="z", name="zps")
                mmr(zps[0:64], w2x[:, pg % 4, 0], s["Rw"][:, 0:BL],
                    start=True, stop=True)
                mmr(zps[64:128], w2x[:, pg % 4, 1], s["Rw"][:, BL:2 * BL],
                    start=True, stop=True)
                U = hT.tile([128, BL], bf, tag="U", name="U")
                nc.vector.tensor_tensor(out=U, in0=zps, in1=s["rb"],
                                        op=Alu.mult)
                R2 = hT.tile([128, BL], bf, tag="R2", name="R2")
                nc.vector.tensor_scalar(
                    out=R2, in0=U, scalar1=b2_sb[:, pg:pg + 1],
                    scalar2=0.0, op0=Alu.add, op1=Alu.max)
                s["R2"] = R2

            def stageD(i):
                e = sched[i]
                s = st[i]
                if e["fb"]:
                    blkst[e["blk"]] = hT.tile([128, BL], dt, tag="o3s",
                                              name="o3")
                if e["fg"]:
                    grp[e["gg"]]["o3g"] = hpo.tile([32, BL], dt, tag="o3g",
                                                   name="o3g")
                gd = grp[e["gg"]]
                mmr(gd["o3g"], gd["w3g"][:, e["q"]], s["R2"],
                    start=e["fg"], stop=e["lg"])
                o3 = blkst[e["blk"]]
                if e["lg"]:
                    g, gs, gg = e["g"], e["gs"], e["gg"]
                    nc.scalar.activation(o3[g * 32:g * 32 + gs],
                                         gd["o3g"][0:gs], Act.Identity,
                                         bias=b3_sb[0:gs, gg:gg + 1],
                                         scale=1.0)
                if e["lb"]:
                    bs, t0b = e["bs"], e["blk"] * 128
                    for bc in range(4):
                        tp = hpw.tile([128, 128], dt, tag="tp", name="tp")
                        nc.tensor.transpose(tp[:, 0:bs],
                                            o3[0:bs, ts(bc, 128)],
                                            idnf[0:bs, 0:bs])
                        nc.vector.tensor_copy(
                            out_sb[bc][:, t0b:t0b + bs], tp[:, 0:bs])
                st[i] = {}

            for j in range(min(3, NP)):
                stageP(j)
            stageA(0)
            for i in range(NP):
                if i + 3 < NP:
                    stageP(i + 3)
                if i + 1 < NP:
                    stageA(i + 1)
                stageB(i)
                stageC(i)
                if i - 2 >= 0:
                    stageD(i - 2)
            stageD(NP - 2)
            stageD(NP - 1)
            for bc in range(4):
                nc.gpsimd.dma_start(out=out[ts(bc, 128)], in_=out_sb[bc])

    nc.compile()
    return nc


def kernel(**inputs):
    from concourse.bass_utils import run_bass_kernel_spmd

    in_maps, nb, nrb, bias_idx, row_idx = _prep(inputs)
    if "nc" not in _cache:
        _cache["nc"] = _build(nb, nrb, bias_idx, row_idx)
    nc = _cache["nc"]
    import os
    res = run_bass_kernel_spmd(
        nc, in_maps, core_ids=list(range(NCORES)),
        trace=bool(int(os.environ.get("KTRACE", "0"))))
    _cache["last_result"] = res
    outs = [r["out"] for r in res.results]
    return np.concatenate(outs, axis=0)
